# revision 14
# baseline (speedup 1.0000x reference)
"""Trainium2 Bass kernel for nn_Aggregation (SAN-style position-dependent
3x3 depthwise aggregation with share_planes=8).

  out[n, c, h, w] = sum_k input[n, c, h+dh(k), w+dw(k)] * weight[n, c//8, k, h*W+w]

Sharding: data-parallel over batch N=8 across the 8 NeuronCores (one image
per core, no collectives).

Per-core design (input [256,56,56], weight [32,9,3136] per image):
  - SBUF partition p = q*32 + g: q in 0..3 = 14-row quarter of the image,
    g in 0..31 = weight group. The 8 share-channels of a group live in the
    free dimension, so each weight element is read via a stride-0 broadcast
    AP instead of being replicated.
  - The host pre-packs ONE flat fp16 slab per partition:
      [ x chunk0 (s=0..3, 4x900) | weight (9x784) | identity row (128) |
        x chunk1 (s=4..7, 4x900) ]
    where each 900-elem x block is a zero-padded flat image quarter
    (guard + 16 rows [14 + 2 halo] * 56 cols + guard + pad), so each tap
    (dh, dw) is a single contiguous 784-slice at offset 1 + (dh+1)*56 + dw.
    Column wrap-around reads are neutralized by zeroing the weight's edge
    columns host-side.
  - fp16 storage: DVE tensor_tensor runs in 2x perf mode (needs 16-bit,
    step 1, 4B-aligned APs -> a second, one-element-shifted copy of the x
    slab, built on-chip by ScalarE, gives every tap an even base offset).
  - Compute: the DVE does ALL tap multiplies (2 fp16/cyc/lane, ~31us
    gapless stream — the hard floor).  Offloading taps to gpsimd was tried
    and is a measured dead end: a concurrent Q7 tensor_tensor holds the
    shared DVE/Pool SBUF port for its whole 6us+ instruction and blocks the
    DVE fp16-2x multiply entirely (trace: DVE ops grow 1.7us -> 7us).
    TensorE accumulates the 9 taps into PSUM with identity-stationary
    matmuls (LDWEIGHTS is pulled ahead by PE's reorder window, so the
    per-matmul identity reload is hidden); ScalarE builds the shifted x
    copies and drains PSUM -> SBUF with an fp16 downcast.
  - PE is clock-gated (HAM) at 1.2GHz until ~3.4us of sustained activity:
    dummy matmuls on an 8th scratch PSUM bank during the DMA-arming window
    pre-warm it to 2.4GHz before the first real matmul.
  - DMA (pieces in consumption order per ring): sync streams
    [x0 s0 | x0 s1 | x0 s2:4 | w5,w6 | x1 s4:6], scalar
    [w0 | w2 | w3,w4 | w7,w8 | x1 s6:8], gpsimd-SWDGE [w1 + identity].
    x0 s0 rides alone at the head so the DVE can start ~2us before the
    full x0 lands; taps 0-2 run as a s0/s1/s23 piece ladder to match.
  - Tails: both chunks close PSUM bank pairs progressively behind the
    final tap (chunk 0 in 2 half-share pieces, chunk 1 in 4 per-share
    pieces), drains split ScalarE / VectorE, and chunk 1's output pieces
    stream on 4 queues (sync, scalar, sync, gpsimd-SWDGE).
"""

import numpy as np

N, C, H, W = 8, 256, 56, 56
G, KK, L = 32, 9, 3136          # weight groups, taps, spatial
SHARE = 8                        # C // G
Q = 4                            # row-quarters
RQ = H // Q                      # 14 rows per quarter
LQ = RQ * W                      # 784 pixels per quarter
XA = 900                         # guard + 16*56 + guard + pad (even)

DTYPE = "float16"                # on-chip storage dtype
SPLIT = 2                        # share-axis chunks (overlap DMA/compute)
SC = SHARE // SPLIT              # share-channels per chunk
# tap order: xa-based taps (dw=+-1) first so compute can start before the
# on-chip xb shift-copies finish; xb-based taps (dw=0) last.
TAP_ORDER = [0, 2, 3, 5, 6, 8, 1, 4, 7]
MM = 448                         # matmul free-dim tile (7 * 448 = 3136)

# packed input slab column offsets (fp16 elements per partition), in weight
# consumption order (plane j = TAP_ORDER[j]).
OFF_X0 = 0
OFF_W0 = SC * XA                 # 3600: plane 0
OFF_W1 = OFF_W0 + LQ             # 4384: plane 1
OFF_ID = OFF_W1 + LQ             # 5168
OFF_W2 = OFF_ID + 128            # 5296: planes 2-8
OFF_X1 = OFF_W2 + 7 * LQ         # 10784
SLAB = OFF_X1 + SC * XA          # 14384

_CACHE = {}


def _build():
    import concourse.bacc as bacc
    import concourse.mybir as mybir
    import concourse.tile as tile

    dt = getattr(mybir.dt, DTYPE)

    nc = bacc.Bacc("TRN2", target_bir_lowering=False, debug=False)
    inp = nc.dram_tensor("inp", [128, SLAB], dt, kind="ExternalInput")
    out = nc.dram_tensor("out", [128, SHARE, LQ], dt, kind="ExternalOutput")

    with tile.TileContext(nc) as tc:
        with (
            tc.tile_pool(name="main", bufs=1) as pool,
            tc.tile_pool(name="prod", bufs=4) as ppool,
            tc.tile_pool(name="psum", bufs=1, space="PSUM") as psum_pool,
        ):
            inbuf = pool.tile([128, SLAB], dt)
            xb = pool.tile([128, SHARE, XA - 4], dt)

            # warm the DVE/ACT dispatch paths while waiting on the first
            # DMAs — the first op on a cold engine pays ~0.8us of i-cache /
            # uop-table setup that would otherwise land on the critical path.
            scrv = pool.tile([128, 16], dt)
            nc.vector.memset(scrv[:, 0:8], 0.0)
            nc.vector.tensor_mul(scrv[:, 8:16], scrv[:, 0:8], scrv[:, 0:8])
            nc.scalar.copy(scrv[:, 8:16], scrv[:, 0:8])

            # PE pre-warm: the HAM clock gate holds PE at 1.2GHz until it
            # has been busy ~3.4us.  Dummy matmuls on a scratch PSUM bank
            # during the DMA-arming window un-throttle it before the first
            # real matmul; the small FD=64 tail keeps queue granularity
            # fine so a ready real matmul is never far behind a dummy.
            scr = pool.tile([128, 520], dt)
            nc.gpsimd.memset(scr[:], 0.0)
            warm_psum = psum_pool.tile(
                [128, 512], mybir.dt.float32, name="warm", tag="warm"
            )
            for _ in range(12):
                nc.tensor.matmul(
                    warm_psum[:], scr[:, 0:128], scr[:, 0:512],
                    start=True, stop=True, skip_group_check=True,
                )

            def pe_filler(n=2):
                # tiny dummy matmuls to plug PE micro-idles (keeps the HAM
                # activity monitor from re-throttling the PE clock to
                # 1.2GHz right before the latency-critical closing matmuls)
                for _ in range(n):
                    nc.tensor.matmul(
                        warm_psum[:, 0:64], scr[:, 0:128], scr[:, 0:64],
                        start=True, stop=True, skip_group_check=True,
                    )

            # DMA queue plan (pieces in consumption order per ring; rings
            # stream concurrently and share the ~358GB/s HBM ceiling).
            for eng, a, b in (
                (nc.sync, OFF_X0, XA),                        # x0 s0
                (nc.scalar, OFF_W0, OFF_W1),                  # w0
                (nc.sync, XA, 2 * XA),                        # x0 s1
                (nc.scalar, OFF_W1, OFF_W2),                  # w1 + ident
                (nc.sync, 2 * XA, OFF_W0),                    # x0 s2:4
                (nc.scalar, OFF_W2, OFF_W2 + LQ),             # w2
                (nc.scalar, OFF_W2 + LQ, OFF_W2 + 3 * LQ),    # w3-4
                (nc.sync, OFF_W2 + 3 * LQ, OFF_W2 + 5 * LQ),  # w5-6
                (nc.scalar, OFF_W2 + 5 * LQ, OFF_X1),         # w7-8
                (nc.sync, OFF_X1, OFF_X1 + 1800),             # x1 s4:6
                (nc.scalar, OFF_X1 + 1800, SLAB),             # x1 s6:8
            ):
                eng.dma_start(out=inbuf[:, a:b], in_=inp.ap()[:, a:b])

            xa_views = [
                inbuf[:, OFF_X0 : OFF_X0 + SC * XA].rearrange(
                    "p (s l) -> p s l", s=SC
                ),
                inbuf[:, OFF_X1 : OFF_X1 + SC * XA].rearrange(
                    "p (s l) -> p s l", s=SC
                ),
            ]
            wt01 = inbuf[:, OFF_W0 : OFF_W0 + 2 * LQ].rearrange(
                "p (k l) -> p k l", k=2
            )
            wt2 = inbuf[:, OFF_W2 : OFF_W2 + 7 * LQ].rearrange(
                "p (k l) -> p k l", k=7
            )

            def w_plane(j):
                if j < 2:
                    return wt01[:, j : j + 1, :]
                return wt2[:, j - 2 : j - 1, :]

            ident = inbuf[:, OFF_ID : OFF_ID + 128]

            # xb = x shifted by one element: gives dw=0 taps an even base;
            # built on-chip by the otherwise-idle ScalarE.
            for c in range(SPLIT):
                nc.scalar.copy(
                    xb[:, c * SC : (c + 1) * SC, :], xa_views[c][:, :, 1 : XA - 3]
                )

            def x_ap_for(c, k):
                dh, dw = k // 3 - 1, k % 3 - 1
                if dw == 0:
                    base = (dh + 1) * W      # even; xb = xa shifted by 1
                    return xb[:, c * SC : (c + 1) * SC, base : base + LQ]
                base = 1 + (dh + 1) * W + dw  # even by construction
                return xa_views[c][:, :, base : base + LQ]

            outbuf = pool.tile([128, SHARE, LQ], dt)
            ofl = out.ap().rearrange("p s l -> p (s l)")
            nhalf = (SC * LQ) // MM          # matmul tiles per chunk

            for c in range(SPLIT):
                s0c, s1c = c * SC, (c + 1) * SC
                # PSUM as three 2-bank pair tiles + one single: matmuls
                # target 448-col slices at bank-aligned offsets, drains read
                # whole pairs in one strided AP.
                pairs = [
                    psum_pool.tile(
                        [128, 1024], mybir.dt.float32,
                        name=f"bankp{c}_{p}", tag=f"bankp{p}",
                    )
                    for p in range(3)
                ] + [
                    psum_pool.tile(
                        [128, MM], mybir.dt.float32,
                        name=f"bankp{c}_3", tag=f"bankp3",
                    )
                ]

                def bank_ap(t):
                    if t < 6:
                        o = (t % 2) * 512
                        return pairs[t // 2][:, o : o + MM]
                    return pairs[3][:]

                def drain(eng, t0, t1, ob):
                    """copy banks [t0:t1) (pair-aligned) to ob columns."""
                    for p in range(t0 // 2, (t1 + 1) // 2):
                        if 2 * p + 1 < t1 and p < 3:
                            src = pairs[p][:].rearrange(
                                "q (b x) -> q b x", b=2
                            )[:, :, 0:MM]
                            dst = ob[:, 2 * p * MM : (2 * p + 2) * MM].rearrange(
                                "q (b x) -> q b x", b=2
                            )
                        else:
                            src = pairs[3][:]
                            dst = ob[:, 6 * MM : 7 * MM]
                        if eng is nc.vector:
                            nc.vector.tensor_copy(out=dst, in_=src)
                        else:
                            nc.scalar.copy(dst, src)

                def mult(j, prod, lo, hi, l0=0, l1=LQ):
                    """multiply shares [lo:hi) x cols [l0:l1) of tap j."""
                    prod_s = prod[:].rearrange("p (s l) -> p s l", s=SC)
                    w_ap = w_plane(j).broadcast_to([128, SC, LQ])
                    x_ap = x_ap_for(c, TAP_ORDER[j])
                    nc.vector.tensor_mul(
                        prod_s[:, lo:hi, l0:l1],
                        x_ap[:, lo:hi, l0:l1],
                        w_ap[:, lo:hi, l0:l1],
                    )

                def mms(j, prod, t0, t1, start, stop):
                    """closing matmuls for banks [t0:t1) of tap j's product."""
                    for t in range(t0, t1):
                        nc.tensor.matmul(
                            bank_ap(t), ident,
                            prod[:, t * MM : (t + 1) * MM],
                            start=start, stop=stop, skip_group_check=True,
                        )

                ob = outbuf[:, s0c:s1c, :].rearrange("p s l -> p (s l)")
                col0 = s0c * LQ

                if c == 0:
                    # head ladder: x0 arrives as s0 | s1 | s23 and the
                    # weight planes stream w0 | w1 | w2 concurrently — taps
                    # 0-2 run piece-multiplies in expected arrival order so
                    # the DVE starts as soon as x0 s0 + w0 land.
                    head = [
                        ppool.tile([128, SC * LQ], dt, name=f"hprod{i}")
                        for i in range(3)
                    ]
                    for j in range(3):
                        mult(j, head[j], 0, 1)          # share s0 -> bank 0
                        mms(j, head[j], 0, 1, j == 0, False)
                        mult(j, head[j], 1, 2)          # share s1 -> banks 1,2
                        mms(j, head[j], 1, 3, j == 0, False)
                    for j in range(3):
                        mult(j, head[j], 2, SC)         # s2:4 -> banks 3:7
                        mms(j, head[j], 3, nhalf, j == 0, False)
                    rest = range(3, KK)
                else:
                    rest = range(KK)

                for j in rest:
                    is_final = j == KK - 1
                    if is_final:
                        break   # final tap handled by the tail blocks
                    prod = ppool.tile([128, SC * LQ], dt)
                    mult(j, prod, 0, SC)
                    mms(j, prod, 0, nhalf, c == 1 and j == 0, False)
                    if c == 1 and j >= 5:
                        pe_filler(2)

                j = KK - 1
                prod = ppool.tile([128, SC * LQ], dt)
                if c < SPLIT - 1:
                    # chunk-0 tail: final tap as 2 half-share pieces with
                    # progressive pair closes, drains on ScalarE, output on
                    # the sync queue (hidden under chunk-1 compute).  The
                    # early pair-0 drain also unblocks chunk-1's first
                    # matmuls (PSUM bank reuse) sooner.
                    mult(j, prod, 0, 2)          # cols :1568 -> banks 0-2
                    mms(j, prod, 0, 3, False, True)
                    drain(nc.scalar, 0, 2, ob)
                    nc.sync.dma_start(
                        out=ofl[:, col0 : col0 + 2 * MM],
                        in_=ob[:, 0 : 2 * MM],
                    )
                    mult(j, prod, 2, SC)
                    mms(j, prod, 3, nhalf, False, True)
                    drain(nc.scalar, 2, 4, ob)
                    nc.sync.dma_start(
                        out=ofl[:, col0 + 2 * MM : col0 + 4 * MM],
                        in_=ob[:, 2 * MM : 4 * MM],
                    )
                    drain(nc.scalar, 4, 6, ob)
                    nc.sync.dma_start(
                        out=ofl[:, col0 + 4 * MM : col0 + 6 * MM],
                        in_=ob[:, 4 * MM : 6 * MM],
                    )
                    drain(nc.scalar, 6, nhalf, ob)
                    nc.sync.dma_start(
                        out=ofl[:, col0 + 6 * MM : col0 + nhalf * MM],
                        in_=ob[:, 6 * MM : nhalf * MM],
                    )
                    continue

                # chunk-1 tail: final tap as 5 pieces (s0 | s1 | s2 | s3
                # cols 0:336 | s3 cols 336:784), banks close progressively
                # and the last piece maps EXACTLY to the single bank 6 so
                # the latency-critical closing atom is as small as possible.
                # Drains + output DMAs pipeline behind on 4 queues.
                mult(j, prod, 0, 1)              # cols :784 -> bank 0
                mms(j, prod, 0, 1, False, True)
                pe_filler(1)
                mult(j, prod, 1, 2)              # cols :1568 -> banks 1, 2
                mms(j, prod, 1, 3, False, True)
                drain(nc.scalar, 0, 2, ob)
                nc.sync.dma_start(
                    out=ofl[:, col0 : col0 + 2 * MM],
                    in_=ob[:, 0 : 2 * MM],
                )
                pe_filler(1)
                mult(j, prod, 2, 3)              # cols :2352 -> banks 3, 4
                mms(j, prod, 3, 5, False, True)
                drain(nc.scalar, 2, 4, ob)
                nc.scalar.dma_start(
                    out=ofl[:, col0 + 2 * MM : col0 + 4 * MM],
                    in_=ob[:, 2 * MM : 4 * MM],
                )
                pe_filler(1)
                mult(j, prod, 3, 4, 0, 336)      # cols :2688 -> bank 5
                mms(j, prod, 5, 6, False, True)
                mult(j, prod, 3, 4, 336, LQ)     # cols :3136 -> bank 6
                mms(j, prod, 6, nhalf, False, True)
                # pair2 drain split DVE (bank 4) / ScalarE (bank 5), single
                # on ScalarE; the two final output pieces ride sync + the
                # idle gpsimd-SWDGE queue so desc-gen runs in parallel.
                nc.vector.tensor_copy(
                    out=ob[:, 4 * MM : 5 * MM], in_=pairs[2][:, 0:MM]
                )
                nc.scalar.copy(ob[:, 5 * MM : 6 * MM], pairs[2][:, 512 : 512 + MM])
                drain(nc.scalar, 6, nhalf, ob)
                nc.sync.dma_start(
                    out=ofl[:, col0 + 4 * MM : col0 + 6 * MM],
                    in_=ob[:, 4 * MM : 6 * MM],
                )
                nc.gpsimd.dma_start(
                    out=ofl[:, col0 + 6 * MM : col0 + nhalf * MM],
                    in_=ob[:, 6 * MM : nhalf * MM],
                )

    nc.compile()
    return nc


def _get_nc():
    if "nc" not in _CACHE:
        _CACHE["nc"] = _build()
    return _CACHE["nc"]


def _prep_shards(input, weight):
    np_dt = np.dtype(DTYPE)
    # padded image per (g, s): rows -1..56 zero-padded
    inp = np.asarray(input).reshape(N, G, SHARE, H, W)
    pad = np.zeros((N, G, SHARE, H + 2, W), dtype=np_dt)
    pad[:, :, :, 1 : H + 1, :] = inp
    # x slab: [N, q, g, s, XA]
    xh = np.zeros((N, Q, G, SHARE, XA), dtype=np_dt)
    for q in range(Q):
        xh[:, q, :, :, 1 : 1 + 16 * W] = pad[:, :, :, q * RQ : q * RQ + 16, :].reshape(
            N, G, SHARE, 16 * W
        )
    xh = xh.reshape(N, 128, SHARE, XA)

    # weight: [N, (q g), k, LQ] with out-of-image edge columns zeroed
    wh = np.asarray(weight).astype(np_dt).reshape(N, G, KK, H, W)
    for k in range(KK):
        dwk = k % 3 - 1
        if dwk == -1:
            wh[:, :, k, :, 0] = 0
        elif dwk == 1:
            wh[:, :, k, :, W - 1] = 0
    wh = (
        wh.reshape(N, G, KK, Q, LQ)
        .transpose(0, 3, 1, 2, 4)
        .reshape(N, 128, KK * LQ)
    )

    wh = wh.reshape(N, 128, KK, LQ)[:, :, TAP_ORDER, :]   # consumption order

    slab = np.empty((N, 128, SLAB), dtype=np_dt)
    slab[:, :, OFF_X0:OFF_W0] = xh[:, :, :SC, :].reshape(N, 128, SC * XA)
    slab[:, :, OFF_W0:OFF_ID] = wh[:, :, :2, :].reshape(N, 128, 2 * LQ)
    slab[:, :, OFF_ID:OFF_W2] = np.eye(128, dtype=np_dt)[None]
    slab[:, :, OFF_W2:OFF_X1] = wh[:, :, 2:, :].reshape(N, 128, 7 * LQ)
    slab[:, :, OFF_X1:SLAB] = xh[:, :, SC:, :].reshape(N, 128, SC * XA)
    return [{"inp": np.ascontiguousarray(slab[n])} for n in range(N)]


def _unpack_out(res_list):
    # res: [128, SHARE, LQ] per core -> (N, C, H, W) float32
    o = np.stack([r["out"] for r in res_list], axis=0).astype(np.float32)
    o = o.reshape(N, Q, G, SHARE, LQ).transpose(0, 2, 3, 1, 4)
    return np.ascontiguousarray(o.reshape(N, C, H, W))


def kernel(input, weight):
    from concourse.bass_utils import run_bass_kernel_spmd

    nc = _get_nc()
    in_maps = _prep_shards(input, weight)
    res = run_bass_kernel_spmd(nc, in_maps, core_ids=list(range(N)))
    return _unpack_out(res.results)
